# revision 1
# baseline (speedup 1.0000x reference)
"""BatchTreeEncoder kernel for 8 Trainium2 NeuronCores.

Reference computation (see problem):
    x = emb[tokens] @ Wc + bc                       # [T, 128]
    v[n] = sum_{m in subtree(n)} x[m]               # bottom-up tree sums
    out[b] = max(max_{n in tree b} v[n], 0)         # per-tree channel max

Strategy: data-parallel over trees (64 trees per core). On the host we
compute, from the integer index tensors only, a DFS (preorder) ordering of
each tree. In DFS order every subtree is a contiguous range, so the subtree
sums become  v_E.T = E.T @ A1  where E is the [500,128] matrix of gathered
embedding rows (DFS order) and A1[t,k] = 1 iff t lies in the subtree of k
(a 0/1 matrix uploaded as data, lower-triangular so it is shipped in 4
column-growing strips). A second matmul applies Wc, and a rank-1 matmul
adds size_k * bc. A free-axis reduce_max + clamp finishes each tree.

All floating-point compute happens on-device in fp32; the host only
manipulates integer indices and the 0/1 structure matrix, plus the final
transpose/concat of core outputs.
"""

import sys

for _p in ("/root/.axon_site", "/root/.axon_site/_ro/trn_rl_repo", "/opt/trn_rl_repo"):
    if _p not in sys.path:
        sys.path.append(_p)

import numpy as np

import concourse.bacc as bacc
import concourse.bass as bass
import concourse.mybir as mybir
import concourse.tile as tile
from concourse.bass_utils import run_bass_kernel_spmd

B = 512          # trees
N = 500          # nodes per tree
D = 128          # embed/encode dim
VOCAB = 50000
NCORES = 8
TPC = B // NCORES            # trees per core (64)
KT = 4                       # 128-row K tiles per tree (500 = 3*128 + 116)
KT_ROWS = [128, 128, 128, 116]
STRIP_W = [128, 256, 384, 500]          # A1 strip widths (cols) per K tile
STRIP_OFS = [0, 128, 384, 768]          # col offsets in the packed strip tensor
STRIP_TOT = 1268

F32 = mybir.dt.float32
I32 = mybir.dt.int32


def _dfs_preprocess(tokens, parent):
    """From parent pointers, compute per-tree DFS preorder.

    Returns (tok_dfs [B,N] int32, size_dfs [B,N] int64).
    size_dfs[b,k] = subtree size of the node at DFS position k; in preorder
    the subtree of position k is exactly positions [k, k+size).
    """
    tok2 = tokens.reshape(B, N)
    pl = parent.reshape(B, N) - (np.arange(B, dtype=np.int64)[:, None] * N)
    pl = pl.copy()
    pl[:, 0] = 0
    rows = np.arange(B)

    size = np.ones((B, N), dtype=np.int64)
    for i in range(N - 1, 0, -1):
        size[rows, pl[:, i]] += size[:, i]

    pos = np.zeros((B, N), dtype=np.int64)
    placed = np.zeros((B, N), dtype=np.int64)
    for i in range(1, N):
        p = pl[:, i]
        pos[:, i] = pos[rows, p] + 1 + placed[rows, p]
        placed[rows, p] += size[:, i]

    node_at = np.empty((B, N), dtype=np.int64)
    node_at[rows[:, None], pos] = np.arange(N)[None, :]

    tok_dfs = np.take_along_axis(tok2, node_at, axis=1).astype(np.int32)
    size_dfs = np.take_along_axis(size, node_at, axis=1)
    return tok_dfs, size_dfs


def _build_a1_strips(size_dfs_core):
    """Pack the per-tree subtree indicator strips.

    size_dfs_core: [TPC, N] int64. Output [TPC, 128, STRIP_TOT] fp32 where
    strip kt occupies cols [STRIP_OFS[kt], +STRIP_W[kt]) and holds
    A1[t, k] = 1 iff k <= t < k + size_k for t in K-tile kt (local rows).
    """
    out = np.zeros((TPC, 128, STRIP_TOT), dtype=np.float32)
    for kt in range(KT):
        r = KT_ROWS[kt]
        w = STRIP_W[kt]
        tg = (128 * kt + np.arange(r))[None, :, None]          # [1, r, 1]
        k = np.arange(w)[None, None, :]                        # [1, 1, w]
        e = k + size_dfs_core[:, None, :w]                     # [TPC, 1, w]
        m = (k <= tg) & (tg < e)
        out[:, :r, STRIP_OFS[kt]:STRIP_OFS[kt] + w] = m.astype(np.float32)
    return out


def _build_program():
    nc = bacc.Bacc("TRN2", target_bir_lowering=False, debug=False, num_devices=1)

    emb_t = nc.dram_tensor("emb_t", [VOCAB, D], F32, kind="ExternalInput")
    a1_t = nc.dram_tensor("a1", [TPC, 128, STRIP_TOT], F32, kind="ExternalInput")
    tok_t = nc.dram_tensor("toki", [TPC, 128, KT], I32, kind="ExternalInput")
    siz_t = nc.dram_tensor("sizes", [TPC, 1, N], F32, kind="ExternalInput")
    wc_t = nc.dram_tensor("wc", [D, D], F32, kind="ExternalInput")
    bc_t = nc.dram_tensor("bc", [1, D], F32, kind="ExternalInput")
    out_t = nc.dram_tensor("out", [D, TPC], F32, kind="ExternalOutput")

    with tile.TileContext(nc) as tc:
        with (
            tc.tile_pool(name="const", bufs=1) as const_pool,
            tc.tile_pool(name="a1p", bufs=3) as a1_pool,
            tc.tile_pool(name="ep", bufs=3) as e_pool,
            tc.tile_pool(name="idxp", bufs=3) as idx_pool,
            tc.tile_pool(name="sizp", bufs=3) as siz_pool,
            tc.tile_pool(name="vep", bufs=2) as ve_pool,
            tc.tile_pool(name="pve", bufs=2, space="PSUM") as pve_pool,
            tc.tile_pool(name="pvx", bufs=2, space="PSUM") as pvx_pool,
        ):
            wc_sb = const_pool.tile([D, D], F32)
            nc.sync.dma_start(out=wc_sb[:], in_=wc_t.ap()[:])
            bc_sb = const_pool.tile([1, D], F32)
            nc.sync.dma_start(out=bc_sb[:], in_=bc_t.ap()[:])
            out_sb = const_pool.tile([D, TPC], F32)

            for tr in range(TPC):
                a1_sb = a1_pool.tile([128, STRIP_TOT], F32)
                nc.sync.dma_start(out=a1_sb[:], in_=a1_t.ap()[tr])
                idx_sb = idx_pool.tile([128, KT], I32)
                nc.sync.dma_start(out=idx_sb[:], in_=tok_t.ap()[tr])
                siz_sb = siz_pool.tile([1, N], F32)
                nc.sync.dma_start(out=siz_sb[:], in_=siz_t.ap()[tr])

                e_sb = e_pool.tile([128, KT * D], F32)
                for kt in range(KT):
                    r = KT_ROWS[kt]
                    nc.gpsimd.indirect_dma_start(
                        out=e_sb[:r, kt * D:(kt + 1) * D],
                        out_offset=None,
                        in_=emb_t.ap()[:],
                        in_offset=bass.IndirectOffsetOnAxis(
                            ap=idx_sb[:r, kt:kt + 1], axis=0
                        ),
                    )

                ve_ps = pve_pool.tile([128, N], F32, space="PSUM")
                # K-tile 3 first: its strip spans all N cols, so the
                # start=True write initializes the full accumulation region.
                for j, kt in enumerate([3, 2, 1, 0]):
                    r = KT_ROWS[kt]
                    w = STRIP_W[kt]
                    o = STRIP_OFS[kt]
                    nc.tensor.matmul(
                        out=ve_ps[:, :w],
                        lhsT=e_sb[:r, kt * D:(kt + 1) * D],
                        rhs=a1_sb[:r, o:o + w],
                        start=(j == 0),
                        stop=(j == KT - 1),
                        skip_group_check=True,
                    )

                ve_sb = ve_pool.tile([128, N], F32)
                nc.scalar.copy(out=ve_sb[:], in_=ve_ps[:])

                vx_ps = pvx_pool.tile([128, N], F32, space="PSUM")
                nc.tensor.matmul(
                    out=vx_ps[:], lhsT=wc_sb[:], rhs=ve_sb[:],
                    start=True, stop=False, skip_group_check=True,
                )
                nc.tensor.matmul(
                    out=vx_ps[:], lhsT=bc_sb[:1, :], rhs=siz_sb[:1, :],
                    start=False, stop=True, skip_group_check=True,
                )

                nc.vector.reduce_max(
                    out=out_sb[:, tr:tr + 1], in_=vx_ps[:],
                    axis=mybir.AxisListType.X,
                )

            nc.vector.tensor_scalar_max(out_sb[:], out_sb[:], 0.0)
            nc.sync.dma_start(out=out_t.ap()[:], in_=out_sb[:])

    nc.compile()
    return nc


def kernel(tokens, parent, depth, node2batch, emb, Wc, bc, bs):
    tokens = np.asarray(tokens, dtype=np.int64)
    parent = np.asarray(parent, dtype=np.int64)
    emb = np.ascontiguousarray(np.asarray(emb, dtype=np.float32))
    Wc = np.ascontiguousarray(np.asarray(Wc, dtype=np.float32))
    bc_row = np.ascontiguousarray(np.asarray(bc, dtype=np.float32).reshape(1, D))

    tok_dfs, size_dfs = _dfs_preprocess(tokens, parent)

    in_maps = []
    for c in range(NCORES):
        sl = slice(c * TPC, (c + 1) * TPC)
        tok_c = tok_dfs[sl]                                   # [TPC, N] int32
        toki = np.zeros((TPC, 128, KT), dtype=np.int32)
        for kt in range(KT):
            r = KT_ROWS[kt]
            toki[:, :r, kt] = tok_c[:, 128 * kt:128 * kt + r]
        in_maps.append({
            "emb_t": emb,
            "a1": _build_a1_strips(size_dfs[sl]),
            "toki": toki,
            "sizes": size_dfs[sl].astype(np.float32)[:, None, :],
            "wc": Wc,
            "bc": bc_row,
        })

    nc = _build_program()
    res = run_bass_kernel_spmd(nc, in_maps, core_ids=list(range(NCORES)))

    out = np.empty((B, D), dtype=np.float32)
    for c in range(NCORES):
        out[c * TPC:(c + 1) * TPC] = res.results[c]["out"].T
    return out


def run_profiled(**inputs):
    """Like kernel() but with trace=True; returns (out, exec_time_ns)."""
    import kernel as _self  # noqa
    tokens = np.asarray(inputs["tokens"], dtype=np.int64)
    parent = np.asarray(inputs["parent"], dtype=np.int64)
    emb = np.ascontiguousarray(np.asarray(inputs["emb"], dtype=np.float32))
    Wc = np.ascontiguousarray(np.asarray(inputs["Wc"], dtype=np.float32))
    bc_row = np.ascontiguousarray(
        np.asarray(inputs["bc"], dtype=np.float32).reshape(1, D))

    tok_dfs, size_dfs = _dfs_preprocess(tokens, parent)
    in_maps = []
    for c in range(NCORES):
        sl = slice(c * TPC, (c + 1) * TPC)
        tok_c = tok_dfs[sl]
        toki = np.zeros((TPC, 128, KT), dtype=np.int32)
        for kt in range(KT):
            r = KT_ROWS[kt]
            toki[:, :r, kt] = tok_c[:, 128 * kt:128 * kt + r]
        in_maps.append({
            "emb_t": emb,
            "a1": _build_a1_strips(size_dfs[sl]),
            "toki": toki,
            "sizes": size_dfs[sl].astype(np.float32)[:, None, :],
            "wc": Wc,
            "bc": bc_row,
        })

    nc = _build_program()
    res = run_bass_kernel_spmd(nc, in_maps, core_ids=list(range(NCORES)),
                               trace=True)
    out = np.empty((B, D), dtype=np.float32)
    for c in range(NCORES):
        out[c * TPC:(c + 1) * TPC] = res.results[c]["out"].T
    return out, res.exec_time_ns
